# revision 39
# baseline (speedup 1.0000x reference)
"""Bahdanau attention Trainium2 kernel, data-parallel over batch on 8 NeuronCores.

Computation (per batch b):
    att[t, v]  = tanh(sum_u x[t,u] W1[v,u] + hW2[b,v] + W1_b[v] + W2_b[v])
    score[t]   = sum_v V[v] att[t,v] + V_b
    e[t]       = exp(score[t])            (softmax without max-subtraction:
                                           |score| <= sum|V| + |V_b| ~ 33, safe in f32)
    w[t]       = e[t] / S,  S = sum_t e[t]
    ctx[u]     = sum_t e[t] x[t,u] / S

Device layout choices:
  - main matmul computes att TRANSPOSED: out1[v, t] (v on partitions) with
    lhsT = W1^T chunks [u,v] (float32r: full-rate PE at N>=256) and
    rhs = x^T chunks [u,t]; x^T is prepared on the host.
  - V-reduction: 4 col-tiled bf16 matmuls (tile_position) put partial scores
    at psum partitions 0/32/64/96; a ones-masked matmul folds them to one row.
  - e is reshaped [1,T] -> [128, T/128] chunkwise by SBUF->SBUF DMAs with
    t = p*NJ + j ordering, so e columns serve as matmul lhsT for the context
    accumulation against natural-layout x rows loaded j-strided (bf16).
  - context matmuls are col-tiled the same way; each batch's context phase is
    software-pipelined into the next batch's main phase.
"""

from contextlib import ExitStack

import ml_dtypes
import numpy as np

import concourse.bass as bass
import concourse.tile as tile
from concourse import bacc, mybir
from concourse.bass import _add_dep_helper
from concourse.bass_utils import run_bass_kernel_spmd

F32 = mybir.dt.float32
F32R = mybir.dt.float32r
BF16 = mybir.dt.bfloat16
FP16 = mybir.dt.float16
P = 128

B_FULL, T_FULL, U_FULL = 32, 2048, 1024
N_CORES = 8

# knobs for the local test harness (the grading path leaves these alone)
TRACE = False
LAST = {}


def build_nc(BPC, T, U):
    KC = U // P          # number of 128-wide u/v chunks
    NT = T // 512        # t-chunks for the main matmul
    NJ = T // P          # j-tiles for the context matmul (t = p*NJ + j)
    UCH = min(U, 512)    # u-chunk width for context output
    NU = U // UCH
    G = min(4, KC)       # col-tiles per V-MM round
    NR = KC // G         # V-MM rounds per t-chunk
    GJ = min(4, NJ)      # col-tiles per context round
    NRJ = NJ // GJ       # context rounds

    nc = bacc.Bacc(None, debug=False)

    xT_d = nc.dram_tensor("xT", [BPC, U, T], F32R, kind="ExternalInput")
    x_d = nc.dram_tensor("xb", [BPC, T, U], BF16, kind="ExternalInput")
    hT_d = nc.dram_tensor("hT", [U, BPC], BF16, kind="ExternalInput")
    w1T_d = nc.dram_tensor("w1T", [U, U], F32R, kind="ExternalInput")
    w2T_d = nc.dram_tensor("w2T", [U, U], BF16, kind="ExternalInput")
    bsum_d = nc.dram_tensor("bsum", [P, KC], F32, kind="ExternalInput")
    vT_d = nc.dram_tensor("vT", [P, KC], BF16, kind="ExternalInput")
    vb_d = nc.dram_tensor("vb", [1, 1], F32, kind="ExternalInput")
    ctx_d = nc.dram_tensor("ctx", [BPC, U], F32, kind="ExternalOutput")
    attw_d = nc.dram_tensor("attw", [BPC, T], F32, kind="ExternalOutput")

    Tanh = mybir.ActivationFunctionType.Tanh
    Exp = mybir.ActivationFunctionType.Exp

    with tile.TileContext(nc) as tc:
        with (
            tc.tile_pool(name="consts", bufs=1) as consts,
            tc.tile_pool(name="w1", bufs=1) as w1p,
            tc.tile_pool(name="xt", bufs=16) as xtp,
            tc.tile_pool(name="xn", bufs=NJ + 4) as xnp,
            tc.tile_pool(name="att", bufs=7) as attp,
            tc.tile_pool(name="e", bufs=2) as ep,
            tc.tile_pool(name="outs", bufs=1) as outp,
        ):
            vT_sb = consts.tile([P, KC], BF16)
            nc.sync.dma_start(out=vT_sb, in_=vT_d[:, :])
            bsum_sb = consts.tile([P, KC], F32)
            nc.sync.dma_start(out=bsum_sb, in_=bsum_d[:, :])
            vb_sb = consts.tile([1, 1], F32)
            nc.sync.dma_start(out=vb_sb, in_=vb_d[:, :])
            hT_sb = consts.tile([P, KC, BPC], BF16)
            nc.sync.dma_start(out=hT_sb, in_=hT_d.rearrange("(k p) b -> p k b", p=P))
            cT_sb = consts.tile([P, KC, BPC], F32)
            # ones at partitions 0,32,... reduce col-tiled psum partials with
            # one matmul instead of a single-lane DVE add chain
            ones_j = consts.tile([P, 1], BF16)
            nc.vector.memset(ones_j, 0.0)
            for i in range(GJ):
                nc.vector.memset(ones_j[32 * i : 32 * i + 1, :], 1.0)
            ones_v = consts.tile([P, 1], FP16)
            nc.vector.memset(ones_v, 0.0)
            for i in range(G):
                nc.vector.memset(ones_v[32 * i : 32 * i + 1, :], 1.0)

            # prologue: cT[p, c, b] = (hidden_b @ W2^T)[c*128+p] + W1_b + W2_b
            # W2^T streams in k-slabs; all KC psum accumulators live at once
            # (prologue-only, so the full 8 banks are free).
            with (
                tc.tile_pool(name="w2", bufs=2) as w2p,
                tc.tile_pool(name="psH", bufs=1, space=bass.MemorySpace.PSUM) as psh,
            ):
                phs = [
                    psh.tile([P, BPC], F32, name=f"ph{c}", tag=f"ph{c}")
                    for c in range(KC)
                ]
                for k in range(KC):
                    w2s = w2p.tile([P, U], BF16)
                    nc.sync.dma_start(
                        out=w2s, in_=w2T_d[k * P : (k + 1) * P, :]
                    )
                    for c in range(KC):
                        nc.tensor.matmul(
                            phs[c],
                            (w2s[:, c * P : (c + 1) * P]),
                            (hT_sb[:, k, :]),
                            start=(k == 0),
                            stop=(k == KC - 1),
                        )
                for c in range(KC):
                    nc.vector.tensor_scalar_add(
                        cT_sb[:, c, :], phs[c], bsum_sb[:, c : c + 1]
                    )

            # batch-0 t-chunk-0 x loads, then the W1 slabs: this DMA order
            # lets the first main matmuls start as soon as slab 0 lands
            # (the first t-chunk runs k-outer), instead of waiting for the
            # whole 4MB of W1 behind the prologue stream.
            xk00 = []
            for k in range(KC):
                xk = xtp.tile([P, 512], F32R, name="xk", tag="xk")
                nc.sync.dma_start(out=xk, in_=xT_d[0, k * P : (k + 1) * P, 0:512])
                xk00.append(xk)
            w1_sb = w1p.tile([P, KC, U], F32R)
            w1_last = None
            for k in range(KC):
                w1_last = nc.sync.dma_start(
                    out=w1_sb[:, k, :], in_=w1T_d[k * P : (k + 1) * P, :]
                )

            with (
                tc.tile_pool(name="psA", bufs=4, space=bass.MemorySpace.PSUM) as ps_main,
                tc.tile_pool(name="psS", bufs=2, space=bass.MemorySpace.PSUM) as ps_sc,
                tc.tile_pool(name="psC", bufs=1, space=bass.MemorySpace.PSUM) as ps_ctx,
            ):
                pe_defer = []         # PE work emitted one v-chunk late so it
                                      # never stalls on a just-issued tanh

                def emit_main_tchunk(b, t, st, xks=None):
                    """xk loads + main MMs + col-tiled V-MMs + exp for one
                    t-chunk; xn prefetch for the context phase is interleaved
                    so the first t-chunk's xk loads are never queued behind
                    it."""
                    if xks is None:
                        xks = []
                        for k in range(KC):
                            xk = xtp.tile([P, 512], F32R)
                            dma = nc.sync.dma_start(
                                out=xk,
                                in_=xT_d[b, k * P : (k + 1) * P,
                                         t * 512 : (t + 1) * 512],
                            )
                            if b == 0 and t == 1:
                                # don't steal HBM bandwidth from the W1 load
                                _add_dep_helper(dma.ins, w1_last.ins, sync=True,
                                                reason="stagger xk behind W1")
                            xks.append(xk)
                    for j in range(t * NJ // NT, (t + 1) * NJ // NT):
                        xn = xnp.tile([P, U], BF16)
                        dma = nc.sync.dma_start(
                            out=xn, in_=x_d[b].rearrange("(p j) u -> j p u", j=NJ)[j]
                        )
                        if b == 0:
                            _add_dep_helper(dma.ins, w1_last.ins, sync=True,
                                            reason="stagger xn behind W1")
                        st["xn"].append(xn)
                    # quad-partial scores: col-tile i writes partition 32*i
                    sc4 = ps_sc.tile([P, 512], F32)
                    atts = []

                    def v_round(r):
                        for i in range(G):
                            nc.tensor.matmul(
                                sc4[32 * i : 32 * i + 1, :],
                                vT_sb[:, r * G + i : r * G + i + 1],
                                atts[r * G + i],
                                start=(r == 0),
                                stop=(r == NR - 1),
                                tile_position=(0, 32 * i),
                            )

                    def emit_v_chunk_tail(v):
                        if pe_defer:
                            pe_defer.pop(0)()
                        if (v + 1) % G == 0:
                            r = (v + 1) // G - 1
                            pe_defer.append(lambda r=r: v_round(r))

                    for v in range(KC):
                        o1 = ps_main.tile([P, 512], F32, name="o1", tag="o1")
                        for k in range(KC):
                            nc.tensor.matmul(
                                o1,
                                w1_sb[:, k, v * P : (v + 1) * P],
                                xks[k],
                                start=(k == 0),
                                stop=(k == KC - 1),
                            )
                        att = attp.tile([P, 512], BF16, name="att", tag="att")
                        nc.scalar.activation(
                            att, o1, Tanh, bias=cT_sb[:, v, b : b + 1]
                        )
                        atts.append(att)
                        emit_v_chunk_tail(v)

                    def reduce_and_exp():
                        # fold the G partial score rows (DVE; off the PE
                        # critical path), then exponentiate
                        s_sb = ep.tile([1, 512], F32, name="s_sb", tag="s_sb")
                        nc.vector.tensor_copy(s_sb, sc4[0:1, :])
                        for i in range(1, G):
                            nc.vector.tensor_add(
                                s_sb, s_sb, sc4[32 * i : 32 * i + 1, :]
                            )
                        nc.scalar.activation(
                            st["e_row"][:, t * 512 : (t + 1) * 512],
                            s_sb,
                            Exp,
                            bias=vb_sb[0:1, :],
                            accum_out=st["s_part"][0:1, t : t + 1],
                        )
                        # incremental e reshape: this t-chunk covers rows
                        # 512//NJ * t .. of e_col (t' = p*NJ + j)
                        RPC = 512 // NJ
                        nc.sync.dma_start(
                            out=st["e_col"][RPC * t : RPC * (t + 1), :],
                            in_=st["e_row"][
                                :, t * 512 : (t + 1) * 512
                            ].rearrange("a (p j) -> a p j", j=NJ),
                        )

                    pe_defer.append(reduce_and_exp)

                def emit_tail(b, st):
                    """softmax: S, 1/S, normalized weights out, e reshaped."""
                    for fn in pe_defer:
                        fn()
                    pe_defer.clear()
                    s_tot = ep.tile([1, 1], F32)
                    nc.vector.tensor_reduce(
                        s_tot, st["s_part"][0:1, :], axis=mybir.AxisListType.X,
                        op=mybir.AluOpType.add,
                    )
                    rec = ep.tile([1, 1], F32)
                    nc.vector.reciprocal(rec, s_tot)
                    st["rec"] = rec

                    w_sb = outp.tile([1, T], F32)
                    nc.vector.tensor_scalar_mul(
                        w_sb, st["e_row"][0:1, :], rec[0:1, 0:1]
                    )
                    nc.sync.dma_start(out=attw_d[b : b + 1, :], in_=w_sb)


                def emit_ctx(b, st):
                    """context: ctx[u] = (sum_t e[t] x[t,u]) / S, with 4
                    j-tiles col-tiled per round (partials at partition 32*i)"""
                    cps = [
                        ps_ctx.tile([P, UCH], F32, name=f"cps{uh}", tag=f"cps{uh}")
                        for uh in range(NU)
                    ]
                    for uh in range(NU):
                        nc.vector.memset(cps[uh], 0.0)
                    for r in range(NRJ):
                        for uh in range(NU):
                            for i in range(GJ):
                                j = r * GJ + i
                                nc.tensor.matmul(
                                    cps[uh][32 * i : 32 * i + 1, :],
                                    st["e_col"][:, j : j + 1],
                                    st["xn"][j][:, uh * UCH : (uh + 1) * UCH],
                                    start=(r == 0),
                                    stop=(r == NRJ - 1),
                                    tile_position=(0, 32 * i),
                                )
                    ctx_sb = outp.tile([1, U], F32)
                    for uh in range(NU):
                        cp_sb = attp.tile(
                            [P, UCH], BF16, name="cp_sb", tag="cp_sb", bufs=2
                        )
                        nc.vector.tensor_copy(cp_sb, cps[uh])
                        nc.tensor.matmul(
                            cps[uh][0:1, :], ones_j, cp_sb, start=True, stop=True
                        )
                        cs = ctx_sb[0:1, uh * UCH : (uh + 1) * UCH]
                        nc.vector.tensor_scalar_mul(
                            cs, cps[uh][0:1, :], st["rec"][0:1, 0:1]
                        )
                    nc.sync.dma_start(out=ctx_d[b : b + 1, :], in_=ctx_sb)

                prev = None
                for b in range(BPC):
                    st = {
                        "xn": [],
                        "e_row": ep.tile([1, T], BF16, name="e_row", tag="e_row"),
                        "e_col": ep.tile([P, NJ], BF16, name="e_col", tag="e_col"),
                        "s_part": ep.tile([1, NT], F32, name="s_part", tag="s_part"),
                    }
                    for t in range(NT):
                        emit_main_tchunk(
                            b, t, st,
                            xks=xk00 if (b == 0 and t == 0) else None,
                        )
                        if t == 0 and prev is not None:
                            # previous batch's context MMs slot in here, where
                            # their e_col/xn dependencies are long satisfied
                            emit_ctx(b - 1, prev)
                            prev = None
                    emit_tail(b, st)
                    prev = st
                for fn in pe_defer:
                    fn()
                pe_defer.clear()
                emit_ctx(BPC - 1, prev)

    nc.compile()
    return nc


_NC_CACHE = {}


def _get_nc(BPC, T, U):
    key = (BPC, T, U)
    if key not in _NC_CACHE:
        _NC_CACHE[key] = build_nc(BPC, T, U)
    return _NC_CACHE[key]


def _prep_shared(W1_w, W1_b, W2_w, W2_b, V_w, V_b, U):
    KC = U // P
    w1T = np.ascontiguousarray(W1_w.T)
    w2T = np.ascontiguousarray(W2_w.T).astype(ml_dtypes.bfloat16)
    bsum = np.ascontiguousarray((W1_b + W2_b).reshape(KC, P).T)
    vT = np.ascontiguousarray(V_w.reshape(KC, P).T).astype(ml_dtypes.bfloat16)
    vb = np.asarray(V_b, np.float32).reshape(1, 1)
    return w1T, w2T, bsum, vT, vb


def kernel(x, hidden, W1_w, W1_b, W2_w, W2_b, V_w, V_b):
    x = np.asarray(x, np.float32)
    hidden = np.asarray(hidden, np.float32)
    W1_w = np.asarray(W1_w, np.float32)
    W1_b = np.asarray(W1_b, np.float32)
    W2_w = np.asarray(W2_w, np.float32)
    W2_b = np.asarray(W2_b, np.float32)
    V_w = np.asarray(V_w, np.float32)
    V_b = np.asarray(V_b, np.float32)

    B, T, U = x.shape
    BPC = B // N_CORES
    nc = _get_nc(BPC, T, U)
    w1T, w2T, bsum, vT, vb = _prep_shared(W1_w, W1_b, W2_w, W2_b, V_w, V_b, U)

    in_maps = []
    for c in range(N_CORES):
        xs = x[c * BPC : (c + 1) * BPC]
        in_maps.append(
            {
                "xb": np.ascontiguousarray(xs).astype(ml_dtypes.bfloat16),
                "xT": np.ascontiguousarray(xs.transpose(0, 2, 1)),
                "hT": np.ascontiguousarray(hidden[c * BPC : (c + 1) * BPC].T).astype(ml_dtypes.bfloat16),
                "w1T": w1T,
                "w2T": w2T,
                "bsum": bsum,
                "vT": vT,
                "vb": vb,
            }
        )

    res = run_bass_kernel_spmd(nc, in_maps, list(range(N_CORES)), trace=TRACE)
    LAST["exec_time_ns"] = res.exec_time_ns
    LAST["results"] = res
    outs = res.results
    ctx = np.concatenate([r["ctx"] for r in outs], axis=0)[None, :, :]
    attw = np.concatenate([r["attw"] for r in outs], axis=0)[:, None, :]
    return ctx.astype(np.float32), attw.astype(np.float32)


# revision 41
# speedup vs baseline: 1.0173x; 1.0173x over previous
"""Bahdanau attention Trainium2 kernel, data-parallel over batch on 8 NeuronCores.

Computation (per batch b):
    att[t, v]  = tanh(sum_u x[t,u] W1[v,u] + hW2[b,v] + W1_b[v] + W2_b[v])
    score[t]   = sum_v V[v] att[t,v] + V_b
    e[t]       = exp(score[t])            (softmax without max-subtraction:
                                           |score| <= sum|V| + |V_b| ~ 33, safe in f32)
    w[t]       = e[t] / S,  S = sum_t e[t]
    ctx[u]     = sum_t e[t] x[t,u] / S

Device layout choices:
  - main matmul computes att TRANSPOSED: out1[v, t] (v on partitions) with
    lhsT = W1^T chunks [u,v] (float32r: full-rate PE at N>=256) and
    rhs = x^T chunks [u,t]; x^T is prepared on the host.
  - V-reduction: 4 col-tiled bf16 matmuls (tile_position) put partial scores
    at psum partitions 0/32/64/96; a ones-masked matmul folds them to one row.
  - e is reshaped [1,T] -> [128, T/128] chunkwise by SBUF->SBUF DMAs with
    t = p*NJ + j ordering, so e columns serve as matmul lhsT for the context
    accumulation against natural-layout x rows loaded j-strided (bf16).
  - context matmuls are col-tiled the same way; each batch's context phase is
    software-pipelined into the next batch's main phase.
"""

import os
from contextlib import ExitStack

import ml_dtypes
import numpy as np

import concourse.bass as bass
import concourse.tile as tile
from concourse import bacc, mybir
from concourse.bass_utils import run_bass_kernel_spmd

F32 = mybir.dt.float32
F32R = mybir.dt.float32r
BF16 = mybir.dt.bfloat16
FP16 = mybir.dt.float16
P = 128

B_FULL, T_FULL, U_FULL = 32, 2048, 1024
N_CORES = 8

# knobs for the local test harness (the grading path leaves these alone)
TRACE = False
LAST = {}


def build_nc(BPC, T, U):
    KC = U // P          # number of 128-wide u/v chunks
    NT = T // 512        # t-chunks for the main matmul
    NJ = T // P          # j-tiles for the context matmul (t = p*NJ + j)
    UCH = min(U, 512)    # u-chunk width for context output
    NU = U // UCH
    G = min(4, KC)       # col-tiles per V-MM round
    NR = KC // G         # V-MM rounds per t-chunk
    GJ = min(4, NJ)      # col-tiles per context round
    NRJ = NJ // GJ       # context rounds

    nc = bacc.Bacc(None, debug=False)

    xT_d = nc.dram_tensor("xT", [BPC, U, T], F32R, kind="ExternalInput")
    x_d = nc.dram_tensor("xb", [BPC, T, U], BF16, kind="ExternalInput")
    hT_d = nc.dram_tensor("hT", [U, BPC], BF16, kind="ExternalInput")
    w1T_d = nc.dram_tensor("w1T", [U, U], F32R, kind="ExternalInput")
    w2T_d = nc.dram_tensor("w2T", [U, U], BF16, kind="ExternalInput")
    bsum_d = nc.dram_tensor("bsum", [P, KC], F32, kind="ExternalInput")
    vT_d = nc.dram_tensor("vT", [P, KC], BF16, kind="ExternalInput")
    vb_d = nc.dram_tensor("vb", [1, 1], F32, kind="ExternalInput")
    ctx_d = nc.dram_tensor("ctx", [BPC, U], F32, kind="ExternalOutput")
    attw_d = nc.dram_tensor("attw", [BPC, T], F32, kind="ExternalOutput")

    Tanh = mybir.ActivationFunctionType.Tanh
    Exp = mybir.ActivationFunctionType.Exp

    with tile.TileContext(nc) as tc:
        with (
            tc.tile_pool(name="consts", bufs=1) as consts,
            tc.tile_pool(name="w1", bufs=1) as w1p,
            tc.tile_pool(name="xt", bufs=16) as xtp,
            tc.tile_pool(name="xn", bufs=NJ + 4) as xnp,
            tc.tile_pool(name="att", bufs=7) as attp,
            tc.tile_pool(name="e", bufs=2) as ep,
            tc.tile_pool(name="outs", bufs=1) as outp,
        ):
            vT_sb = consts.tile([P, KC], BF16)
            nc.sync.dma_start(out=vT_sb, in_=vT_d[:, :])
            bsum_sb = consts.tile([P, KC], F32)
            nc.sync.dma_start(out=bsum_sb, in_=bsum_d[:, :])
            vb_sb = consts.tile([1, 1], F32)
            nc.sync.dma_start(out=vb_sb, in_=vb_d[:, :])
            hT_sb = consts.tile([P, KC, BPC], BF16)
            nc.sync.dma_start(out=hT_sb, in_=hT_d.rearrange("(k p) b -> p k b", p=P))
            cT_sb = consts.tile([P, KC, BPC], F32)
            # ones at partitions 0,32,... reduce col-tiled psum partials with
            # one matmul instead of a single-lane DVE add chain
            ones_j = consts.tile([P, 1], BF16)
            nc.vector.memset(ones_j, 0.0)
            for i in range(GJ):
                nc.vector.memset(ones_j[32 * i : 32 * i + 1, :], 1.0)
            ones_v = consts.tile([P, 1], FP16)
            nc.vector.memset(ones_v, 0.0)
            for i in range(G):
                nc.vector.memset(ones_v[32 * i : 32 * i + 1, :], 1.0)

            # prologue: cT[p, c, b] = (hidden_b @ W2^T)[c*128+p] + W1_b + W2_b
            # W2^T streams in k-slabs; all KC psum accumulators live at once
            # (prologue-only, so the full 8 banks are free).
            with (
                tc.tile_pool(name="w2", bufs=2) as w2p,
                tc.tile_pool(name="psH", bufs=1, space=bass.MemorySpace.PSUM) as psh,
            ):
                phs = [
                    psh.tile([P, BPC], F32, name=f"ph{c}", tag=f"ph{c}")
                    for c in range(KC)
                ]
                for k in range(KC):
                    w2s = w2p.tile([P, U], BF16)
                    nc.sync.dma_start(
                        out=w2s, in_=w2T_d[k * P : (k + 1) * P, :]
                    )
                    for c in range(KC):
                        nc.tensor.matmul(
                            phs[c],
                            (w2s[:, c * P : (c + 1) * P]),
                            (hT_sb[:, k, :]),
                            start=(k == 0),
                            stop=(k == KC - 1),
                        )
                for c in range(KC):
                    nc.vector.tensor_scalar_add(
                        cT_sb[:, c, :], phs[c], bsum_sb[:, c : c + 1]
                    )

            # batch-0 t-chunk-0 x loads, then the W1 slabs: this DMA order
            # lets the first main matmuls start as soon as slab 0 lands
            # (the first t-chunk runs k-outer), instead of waiting for the
            # whole 4MB of W1 behind the prologue stream.
            xk00 = []
            for k in range(KC):
                xk = xtp.tile([P, 512], F32R, name="xk", tag="xk")
                nc.sync.dma_start(out=xk, in_=xT_d[0, k * P : (k + 1) * P, 0:512])
                xk00.append(xk)
            w1_sb = w1p.tile([P, KC, U], F32R)
            for k in range(KC):
                nc.sync.dma_start(
                    out=w1_sb[:, k, :], in_=w1T_d[k * P : (k + 1) * P, :]
                )

            with (
                tc.tile_pool(name="psA", bufs=4, space=bass.MemorySpace.PSUM) as ps_main,
                tc.tile_pool(name="psS", bufs=2, space=bass.MemorySpace.PSUM) as ps_sc,
                tc.tile_pool(name="psC", bufs=1, space=bass.MemorySpace.PSUM) as ps_ctx,
            ):
                pe_defer = []         # PE work emitted one v-chunk late so it
                                      # never stalls on a just-issued tanh

                def emit_main_tchunk(b, t, st, xks=None):
                    """xk loads + main MMs + col-tiled V-MMs + exp for one
                    t-chunk; xn prefetch for the context phase is interleaved
                    so the first t-chunk's xk loads are never queued behind
                    it."""
                    if xks is None:
                        xks = []
                        for k in range(KC):
                            xk = xtp.tile([P, 512], F32R)
                            nc.sync.dma_start(
                                out=xk,
                                in_=xT_d[b, k * P : (k + 1) * P,
                                         t * 512 : (t + 1) * 512],
                            )
                            xks.append(xk)
                    for j in range(t * NJ // NT, (t + 1) * NJ // NT):
                        xn = xnp.tile([P, U], BF16)
                        nc.sync.dma_start(
                            out=xn, in_=x_d[b].rearrange("(p j) u -> j p u", j=NJ)[j]
                        )
                        st["xn"].append(xn)
                    # quad-partial scores: col-tile i writes partition 32*i
                    sc4 = ps_sc.tile([P, 512], F32)
                    atts = []

                    def v_round(r):
                        for i in range(G):
                            nc.tensor.matmul(
                                sc4[32 * i : 32 * i + 1, :],
                                vT_sb[:, r * G + i : r * G + i + 1],
                                atts[r * G + i],
                                start=(r == 0),
                                stop=(r == NR - 1),
                                tile_position=(0, 32 * i),
                            )

                    def emit_v_chunk_tail(v):
                        if pe_defer:
                            pe_defer.pop(0)()
                        if (v + 1) % G == 0:
                            r = (v + 1) // G - 1
                            pe_defer.append(lambda r=r: v_round(r))

                    for v in range(KC):
                        o1 = ps_main.tile([P, 512], F32, name="o1", tag="o1")
                        for k in range(KC):
                            nc.tensor.matmul(
                                o1,
                                w1_sb[:, k, v * P : (v + 1) * P],
                                xks[k],
                                start=(k == 0),
                                stop=(k == KC - 1),
                            )
                        att = attp.tile([P, 512], BF16, name="att", tag="att")
                        nc.scalar.activation(
                            att, o1, Tanh, bias=cT_sb[:, v, b : b + 1]
                        )
                        atts.append(att)
                        emit_v_chunk_tail(v)

                    def reduce_and_exp():
                        # fold the G partial score rows (DVE; off the PE
                        # critical path), then exponentiate
                        s_sb = ep.tile([1, 512], F32, name="s_sb", tag="s_sb")
                        nc.vector.tensor_copy(s_sb, sc4[0:1, :])
                        for i in range(1, G):
                            nc.vector.tensor_add(
                                s_sb, s_sb, sc4[32 * i : 32 * i + 1, :]
                            )
                        nc.scalar.activation(
                            st["e_row"][:, t * 512 : (t + 1) * 512],
                            s_sb,
                            Exp,
                            bias=vb_sb[0:1, :],
                            accum_out=st["s_part"][0:1, t : t + 1],
                        )
                        # incremental e reshape: this t-chunk covers rows
                        # 512//NJ * t .. of e_col (t' = p*NJ + j)
                        RPC = 512 // NJ
                        nc.sync.dma_start(
                            out=st["e_col"][RPC * t : RPC * (t + 1), :],
                            in_=st["e_row"][
                                :, t * 512 : (t + 1) * 512
                            ].rearrange("a (p j) -> a p j", j=NJ),
                        )

                    pe_defer.append(reduce_and_exp)

                def emit_tail(b, st):
                    """softmax: S, 1/S, normalized weights out, e reshaped."""
                    for fn in pe_defer:
                        fn()
                    pe_defer.clear()
                    s_tot = ep.tile([1, 1], F32)
                    nc.vector.tensor_reduce(
                        s_tot, st["s_part"][0:1, :], axis=mybir.AxisListType.X,
                        op=mybir.AluOpType.add,
                    )
                    rec = ep.tile([1, 1], F32)
                    nc.vector.reciprocal(rec, s_tot)
                    st["rec"] = rec

                    w_sb = outp.tile([1, T], F32)
                    nc.vector.tensor_scalar_mul(
                        w_sb, st["e_row"][0:1, :], rec[0:1, 0:1]
                    )
                    nc.sync.dma_start(out=attw_d[b : b + 1, :], in_=w_sb)


                def emit_ctx(b, st):
                    """context: ctx[u] = (sum_t e[t] x[t,u]) / S, with 4
                    j-tiles col-tiled per round (partials at partition 32*i)"""
                    cps = [
                        ps_ctx.tile([P, UCH], F32, name=f"cps{uh}", tag=f"cps{uh}")
                        for uh in range(NU)
                    ]
                    for uh in range(NU):
                        nc.vector.memset(cps[uh], 0.0)
                    for r in range(NRJ):
                        for uh in range(NU):
                            for i in range(GJ):
                                j = r * GJ + i
                                nc.tensor.matmul(
                                    cps[uh][32 * i : 32 * i + 1, :],
                                    st["e_col"][:, j : j + 1],
                                    st["xn"][j][:, uh * UCH : (uh + 1) * UCH],
                                    start=(r == 0),
                                    stop=(r == NRJ - 1),
                                    tile_position=(0, 32 * i),
                                )
                    ctx_sb = outp.tile([1, U], F32)
                    for uh in range(NU):
                        cp_sb = attp.tile(
                            [P, UCH], BF16, name="cp_sb", tag="cp_sb", bufs=2
                        )
                        nc.vector.tensor_copy(cp_sb, cps[uh])
                        nc.tensor.matmul(
                            cps[uh][0:1, :], ones_j, cp_sb, start=True, stop=True
                        )
                        cs = ctx_sb[0:1, uh * UCH : (uh + 1) * UCH]
                        nc.vector.tensor_scalar_mul(
                            cs, cps[uh][0:1, :], st["rec"][0:1, 0:1]
                        )
                    nc.sync.dma_start(out=ctx_d[b : b + 1, :], in_=ctx_sb)

                prev = None
                for b in range(BPC):
                    st = {
                        "xn": [],
                        "e_row": ep.tile([1, T], BF16, name="e_row", tag="e_row"),
                        "e_col": ep.tile([P, NJ], BF16, name="e_col", tag="e_col"),
                        "s_part": ep.tile([1, NT], F32, name="s_part", tag="s_part"),
                    }
                    for t in range(NT):
                        emit_main_tchunk(
                            b, t, st,
                            xks=xk00 if (b == 0 and t == 0) else None,
                        )
                        if t == 0 and prev is not None:
                            # previous batch's context MMs slot in here, where
                            # their e_col/xn dependencies are long satisfied
                            emit_ctx(b - 1, prev)
                            prev = None
                    emit_tail(b, st)
                    prev = st
                for fn in pe_defer:
                    fn()
                pe_defer.clear()
                emit_ctx(BPC - 1, prev)

    nc.compile()
    return nc


_NC_CACHE = {}


def _get_nc(BPC, T, U):
    key = (BPC, T, U)
    if key not in _NC_CACHE:
        _NC_CACHE[key] = build_nc(BPC, T, U)
    return _NC_CACHE[key]


def _prep_shared(W1_w, W1_b, W2_w, W2_b, V_w, V_b, U):
    KC = U // P
    w1T = np.ascontiguousarray(W1_w.T)
    w2T = np.ascontiguousarray(W2_w.T).astype(ml_dtypes.bfloat16)
    bsum = np.ascontiguousarray((W1_b + W2_b).reshape(KC, P).T)
    vT = np.ascontiguousarray(V_w.reshape(KC, P).T).astype(ml_dtypes.bfloat16)
    vb = np.asarray(V_b, np.float32).reshape(1, 1)
    return w1T, w2T, bsum, vT, vb


def kernel(x, hidden, W1_w, W1_b, W2_w, W2_b, V_w, V_b):
    x = np.asarray(x, np.float32)
    hidden = np.asarray(hidden, np.float32)
    W1_w = np.asarray(W1_w, np.float32)
    W1_b = np.asarray(W1_b, np.float32)
    W2_w = np.asarray(W2_w, np.float32)
    W2_b = np.asarray(W2_b, np.float32)
    V_w = np.asarray(V_w, np.float32)
    V_b = np.asarray(V_b, np.float32)

    if TRACE:
        os.environ.pop("BASS_NEVER_TRACE", None)
    else:
        # the axon trace path needs a profiling hook this image may lack;
        # make sure an ambient BASS_TRACE can't route us into it
        os.environ["BASS_NEVER_TRACE"] = "1"

    B, T, U = x.shape
    BPC = B // N_CORES
    nc = _get_nc(BPC, T, U)
    w1T, w2T, bsum, vT, vb = _prep_shared(W1_w, W1_b, W2_w, W2_b, V_w, V_b, U)

    in_maps = []
    for c in range(N_CORES):
        xs = x[c * BPC : (c + 1) * BPC]
        in_maps.append(
            {
                "xb": np.ascontiguousarray(xs).astype(ml_dtypes.bfloat16),
                "xT": np.ascontiguousarray(xs.transpose(0, 2, 1)),
                "hT": np.ascontiguousarray(hidden[c * BPC : (c + 1) * BPC].T).astype(ml_dtypes.bfloat16),
                "w1T": w1T,
                "w2T": w2T,
                "bsum": bsum,
                "vT": vT,
                "vb": vb,
            }
        )

    res = run_bass_kernel_spmd(nc, in_maps, list(range(N_CORES)), trace=TRACE)
    LAST["exec_time_ns"] = res.exec_time_ns
    LAST["results"] = res
    outs = res.results
    ctx = np.concatenate([r["ctx"] for r in outs], axis=0)[None, :, :]
    attw = np.concatenate([r["attw"] for r in outs], axis=0)[:, None, :]
    return ctx.astype(np.float32), attw.astype(np.float32)


# revision 42
# speedup vs baseline: 1.0176x; 1.0003x over previous
"""Bahdanau attention Trainium2 kernel, data-parallel over batch on 8 NeuronCores.

Computation (per batch b):
    att[t, v]  = tanh(sum_u x[t,u] W1[v,u] + hW2[b,v] + W1_b[v] + W2_b[v])
    score[t]   = sum_v V[v] att[t,v] + V_b
    e[t]       = exp(score[t])            (softmax without max-subtraction:
                                           |score| <= sum|V| + |V_b| ~ 33, safe in f32)
    w[t]       = e[t] / S,  S = sum_t e[t]
    ctx[u]     = sum_t e[t] x[t,u] / S

Device layout choices:
  - main matmul computes att TRANSPOSED: out1[v, t] (v on partitions) with
    lhsT = W1^T chunks [u,v] (float32r: full-rate PE at N>=256) and
    rhs = x^T chunks [u,t]; x^T is prepared on the host.
  - V-reduction: 4 col-tiled bf16 matmuls (tile_position) put partial scores
    at psum partitions 0/32/64/96; a ones-masked matmul folds them to one row.
  - e is reshaped [1,T] -> [128, T/128] chunkwise by SBUF->SBUF DMAs with
    t = p*NJ + j ordering, so e columns serve as matmul lhsT for the context
    accumulation against natural-layout x rows loaded j-strided (bf16).
  - context matmuls are col-tiled the same way; each batch's context phase is
    software-pipelined into the next batch's main phase.
"""

import os
from contextlib import ExitStack

import ml_dtypes
import numpy as np

import concourse.bass as bass
import concourse.tile as tile
from concourse import bacc, mybir
from concourse.bass_utils import run_bass_kernel_spmd

F32 = mybir.dt.float32
F32R = mybir.dt.float32r
BF16 = mybir.dt.bfloat16
FP16 = mybir.dt.float16
P = 128

B_FULL, T_FULL, U_FULL = 32, 2048, 1024
N_CORES = 8

# knobs for the local test harness (the grading path leaves these alone)
TRACE = False
LAST = {}


def build_nc(BPC, T, U):
    KC = U // P          # number of 128-wide u/v chunks
    NT = T // 512        # t-chunks for the main matmul
    NJ = T // P          # j-tiles for the context matmul (t = p*NJ + j)
    UCH = min(U, 512)    # u-chunk width for context output
    NU = U // UCH
    G = min(4, KC)       # col-tiles per V-MM round
    NR = KC // G         # V-MM rounds per t-chunk
    GJ = min(4, NJ)      # col-tiles per context round
    NRJ = NJ // GJ       # context rounds

    nc = bacc.Bacc(None, debug=False)

    xT_d = nc.dram_tensor("xT", [BPC, U, T], F32R, kind="ExternalInput")
    x_d = nc.dram_tensor("xb", [BPC, T, U], BF16, kind="ExternalInput")
    hT_d = nc.dram_tensor("hT", [U, BPC], BF16, kind="ExternalInput")
    w1T_d = nc.dram_tensor("w1T", [U, U], F32R, kind="ExternalInput")
    w2T_d = nc.dram_tensor("w2T", [U, U], BF16, kind="ExternalInput")
    bsum_d = nc.dram_tensor("bsum", [P, KC], F32, kind="ExternalInput")
    vT_d = nc.dram_tensor("vT", [P, KC], BF16, kind="ExternalInput")
    vb_d = nc.dram_tensor("vb", [1, 1], F32, kind="ExternalInput")
    ctx_d = nc.dram_tensor("ctx", [BPC, U], F32, kind="ExternalOutput")
    attw_d = nc.dram_tensor("attw", [BPC, T], F32, kind="ExternalOutput")

    Tanh = mybir.ActivationFunctionType.Tanh
    Exp = mybir.ActivationFunctionType.Exp

    with tile.TileContext(nc) as tc:
        with (
            tc.tile_pool(name="consts", bufs=1) as consts,
            tc.tile_pool(name="w1", bufs=1) as w1p,
            tc.tile_pool(name="xt", bufs=24) as xtp,
            tc.tile_pool(name="xn", bufs=NJ + 6) as xnp,
            tc.tile_pool(name="att", bufs=7) as attp,
            tc.tile_pool(name="e", bufs=2) as ep,
            tc.tile_pool(name="outs", bufs=1) as outp,
        ):
            vT_sb = consts.tile([P, KC], BF16)
            nc.sync.dma_start(out=vT_sb, in_=vT_d[:, :])
            bsum_sb = consts.tile([P, KC], F32)
            nc.sync.dma_start(out=bsum_sb, in_=bsum_d[:, :])
            vb_sb = consts.tile([1, 1], F32)
            nc.sync.dma_start(out=vb_sb, in_=vb_d[:, :])
            hT_sb = consts.tile([P, KC, BPC], BF16)
            nc.sync.dma_start(out=hT_sb, in_=hT_d.rearrange("(k p) b -> p k b", p=P))
            cT_sb = consts.tile([P, KC, BPC], F32)
            # ones at partitions 0,32,... reduce col-tiled psum partials with
            # one matmul instead of a single-lane DVE add chain
            ones_j = consts.tile([P, 1], BF16)
            nc.vector.memset(ones_j, 0.0)
            for i in range(GJ):
                nc.vector.memset(ones_j[32 * i : 32 * i + 1, :], 1.0)
            ones_v = consts.tile([P, 1], FP16)
            nc.vector.memset(ones_v, 0.0)
            for i in range(G):
                nc.vector.memset(ones_v[32 * i : 32 * i + 1, :], 1.0)

            # prologue: cT[p, c, b] = (hidden_b @ W2^T)[c*128+p] + W1_b + W2_b
            # W2^T streams in k-slabs; all KC psum accumulators live at once
            # (prologue-only, so the full 8 banks are free).
            with (
                tc.tile_pool(name="w2", bufs=2) as w2p,
                tc.tile_pool(name="psH", bufs=1, space=bass.MemorySpace.PSUM) as psh,
            ):
                phs = [
                    psh.tile([P, BPC], F32, name=f"ph{c}", tag=f"ph{c}")
                    for c in range(KC)
                ]
                for k in range(KC):
                    w2s = w2p.tile([P, U], BF16)
                    nc.sync.dma_start(
                        out=w2s, in_=w2T_d[k * P : (k + 1) * P, :]
                    )
                    for c in range(KC):
                        nc.tensor.matmul(
                            phs[c],
                            (w2s[:, c * P : (c + 1) * P]),
                            (hT_sb[:, k, :]),
                            start=(k == 0),
                            stop=(k == KC - 1),
                        )
                for c in range(KC):
                    nc.vector.tensor_scalar_add(
                        cT_sb[:, c, :], phs[c], bsum_sb[:, c : c + 1]
                    )

            # batch-0 t-chunk-0 x loads, then the W1 slabs: this DMA order
            # lets the first main matmuls start as soon as slab 0 lands
            # (the first t-chunk runs k-outer), instead of waiting for the
            # whole 4MB of W1 behind the prologue stream.
            xk00 = []
            for k in range(KC):
                xk = xtp.tile([P, 512], F32R, name="xk", tag="xk")
                nc.sync.dma_start(out=xk, in_=xT_d[0, k * P : (k + 1) * P, 0:512])
                xk00.append(xk)
            w1_sb = w1p.tile([P, KC, U], F32R)
            for k in range(KC):
                nc.sync.dma_start(
                    out=w1_sb[:, k, :], in_=w1T_d[k * P : (k + 1) * P, :]
                )

            with (
                tc.tile_pool(name="psA", bufs=4, space=bass.MemorySpace.PSUM) as ps_main,
                tc.tile_pool(name="psS", bufs=2, space=bass.MemorySpace.PSUM) as ps_sc,
                tc.tile_pool(name="psC", bufs=1, space=bass.MemorySpace.PSUM) as ps_ctx,
            ):
                pe_defer = []         # PE work emitted one v-chunk late so it
                                      # never stalls on a just-issued tanh

                def emit_main_tchunk(b, t, st, xks=None):
                    """xk loads + main MMs + col-tiled V-MMs + exp for one
                    t-chunk; xn prefetch for the context phase is interleaved
                    so the first t-chunk's xk loads are never queued behind
                    it."""
                    if xks is None:
                        xks = []
                        for k in range(KC):
                            xk = xtp.tile([P, 512], F32R)
                            nc.sync.dma_start(
                                out=xk,
                                in_=xT_d[b, k * P : (k + 1) * P,
                                         t * 512 : (t + 1) * 512],
                            )
                            xks.append(xk)
                    jlo = (t - 1 if b == 0 else t) * NJ // NT
                    jhi = (t if b == 0 else t + 1) * NJ // NT
                    for j in range(max(jlo, 0), jhi):
                        xn = xnp.tile([P, U], BF16)
                        nc.sync.dma_start(
                            out=xn, in_=x_d[b].rearrange("(p j) u -> j p u", j=NJ)[j]
                        )
                        st["xn"].append(xn)
                    # quad-partial scores: col-tile i writes partition 32*i
                    sc4 = ps_sc.tile([P, 512], F32)
                    atts = []

                    def v_round(r):
                        for i in range(G):
                            nc.tensor.matmul(
                                sc4[32 * i : 32 * i + 1, :],
                                vT_sb[:, r * G + i : r * G + i + 1],
                                atts[r * G + i],
                                start=(r == 0),
                                stop=(r == NR - 1),
                                tile_position=(0, 32 * i),
                            )

                    def emit_v_chunk_tail(v):
                        if pe_defer:
                            pe_defer.pop(0)()
                        if (v + 1) % G == 0:
                            r = (v + 1) // G - 1
                            pe_defer.append(lambda r=r: v_round(r))

                    for v in range(KC):
                        o1 = ps_main.tile([P, 512], F32, name="o1", tag="o1")
                        for k in range(KC):
                            nc.tensor.matmul(
                                o1,
                                w1_sb[:, k, v * P : (v + 1) * P],
                                xks[k],
                                start=(k == 0),
                                stop=(k == KC - 1),
                            )
                        att = attp.tile([P, 512], BF16, name="att", tag="att")
                        nc.scalar.activation(
                            att, o1, Tanh, bias=cT_sb[:, v, b : b + 1]
                        )
                        atts.append(att)
                        emit_v_chunk_tail(v)

                    def reduce_and_exp():
                        # fold the G partial score rows (DVE; off the PE
                        # critical path), then exponentiate
                        s_sb = ep.tile([1, 512], F32, name="s_sb", tag="s_sb")
                        nc.vector.tensor_copy(s_sb, sc4[0:1, :])
                        for i in range(1, G):
                            nc.vector.tensor_add(
                                s_sb, s_sb, sc4[32 * i : 32 * i + 1, :]
                            )
                        nc.scalar.activation(
                            st["e_row"][:, t * 512 : (t + 1) * 512],
                            s_sb,
                            Exp,
                            bias=vb_sb[0:1, :],
                            accum_out=st["s_part"][0:1, t : t + 1],
                        )
                        # incremental e reshape: this t-chunk covers rows
                        # 512//NJ * t .. of e_col (t' = p*NJ + j)
                        RPC = 512 // NJ
                        nc.sync.dma_start(
                            out=st["e_col"][RPC * t : RPC * (t + 1), :],
                            in_=st["e_row"][
                                :, t * 512 : (t + 1) * 512
                            ].rearrange("a (p j) -> a p j", j=NJ),
                        )

                    pe_defer.append(reduce_and_exp)

                def emit_tail(b, st):
                    """softmax: S, 1/S, normalized weights out, e reshaped."""
                    if b == 0:
                        for j in range((NT - 1) * NJ // NT, NJ):
                            xn = xnp.tile([P, U], BF16)
                            nc.sync.dma_start(
                                out=xn,
                                in_=x_d[b].rearrange("(p j) u -> j p u", j=NJ)[j],
                            )
                            st["xn"].append(xn)
                    for fn in pe_defer:
                        fn()
                    pe_defer.clear()
                    s_tot = ep.tile([1, 1], F32)
                    nc.vector.tensor_reduce(
                        s_tot, st["s_part"][0:1, :], axis=mybir.AxisListType.X,
                        op=mybir.AluOpType.add,
                    )
                    rec = ep.tile([1, 1], F32)
                    nc.vector.reciprocal(rec, s_tot)
                    st["rec"] = rec

                    w_sb = outp.tile([1, T], F32)
                    nc.vector.tensor_scalar_mul(
                        w_sb, st["e_row"][0:1, :], rec[0:1, 0:1]
                    )
                    nc.sync.dma_start(out=attw_d[b : b + 1, :], in_=w_sb)


                def emit_ctx(b, st):
                    """context: ctx[u] = (sum_t e[t] x[t,u]) / S, with 4
                    j-tiles col-tiled per round (partials at partition 32*i)"""
                    cps = [
                        ps_ctx.tile([P, UCH], F32, name=f"cps{uh}", tag=f"cps{uh}")
                        for uh in range(NU)
                    ]
                    for uh in range(NU):
                        nc.vector.memset(cps[uh], 0.0)
                    for r in range(NRJ):
                        for uh in range(NU):
                            for i in range(GJ):
                                j = r * GJ + i
                                nc.tensor.matmul(
                                    cps[uh][32 * i : 32 * i + 1, :],
                                    st["e_col"][:, j : j + 1],
                                    st["xn"][j][:, uh * UCH : (uh + 1) * UCH],
                                    start=(r == 0),
                                    stop=(r == NRJ - 1),
                                    tile_position=(0, 32 * i),
                                )
                    ctx_sb = outp.tile([1, U], F32)
                    for uh in range(NU):
                        cp_sb = attp.tile(
                            [P, UCH], BF16, name="cp_sb", tag="cp_sb", bufs=2
                        )
                        nc.vector.tensor_copy(cp_sb, cps[uh])
                        nc.tensor.matmul(
                            cps[uh][0:1, :], ones_j, cp_sb, start=True, stop=True
                        )
                        cs = ctx_sb[0:1, uh * UCH : (uh + 1) * UCH]
                        nc.vector.tensor_scalar_mul(
                            cs, cps[uh][0:1, :], st["rec"][0:1, 0:1]
                        )
                    nc.sync.dma_start(out=ctx_d[b : b + 1, :], in_=ctx_sb)

                prev = None
                for b in range(BPC):
                    st = {
                        "xn": [],
                        "e_row": ep.tile([1, T], BF16, name="e_row", tag="e_row"),
                        "e_col": ep.tile([P, NJ], BF16, name="e_col", tag="e_col"),
                        "s_part": ep.tile([1, NT], F32, name="s_part", tag="s_part"),
                    }
                    for t in range(NT):
                        emit_main_tchunk(
                            b, t, st,
                            xks=xk00 if (b == 0 and t == 0) else None,
                        )
                        if t == 0 and prev is not None:
                            # previous batch's context MMs slot in here, where
                            # their e_col/xn dependencies are long satisfied
                            emit_ctx(b - 1, prev)
                            prev = None
                    emit_tail(b, st)
                    prev = st
                for fn in pe_defer:
                    fn()
                pe_defer.clear()
                emit_ctx(BPC - 1, prev)

    nc.compile()
    return nc


_NC_CACHE = {}


def _get_nc(BPC, T, U):
    key = (BPC, T, U)
    if key not in _NC_CACHE:
        _NC_CACHE[key] = build_nc(BPC, T, U)
    return _NC_CACHE[key]


def _prep_shared(W1_w, W1_b, W2_w, W2_b, V_w, V_b, U):
    KC = U // P
    w1T = np.ascontiguousarray(W1_w.T)
    w2T = np.ascontiguousarray(W2_w.T).astype(ml_dtypes.bfloat16)
    bsum = np.ascontiguousarray((W1_b + W2_b).reshape(KC, P).T)
    vT = np.ascontiguousarray(V_w.reshape(KC, P).T).astype(ml_dtypes.bfloat16)
    vb = np.asarray(V_b, np.float32).reshape(1, 1)
    return w1T, w2T, bsum, vT, vb


def kernel(x, hidden, W1_w, W1_b, W2_w, W2_b, V_w, V_b):
    x = np.asarray(x, np.float32)
    hidden = np.asarray(hidden, np.float32)
    W1_w = np.asarray(W1_w, np.float32)
    W1_b = np.asarray(W1_b, np.float32)
    W2_w = np.asarray(W2_w, np.float32)
    W2_b = np.asarray(W2_b, np.float32)
    V_w = np.asarray(V_w, np.float32)
    V_b = np.asarray(V_b, np.float32)

    if TRACE:
        os.environ.pop("BASS_NEVER_TRACE", None)
    else:
        # the axon trace path needs a profiling hook this image may lack;
        # make sure an ambient BASS_TRACE can't route us into it
        os.environ["BASS_NEVER_TRACE"] = "1"

    B, T, U = x.shape
    BPC = B // N_CORES
    nc = _get_nc(BPC, T, U)
    w1T, w2T, bsum, vT, vb = _prep_shared(W1_w, W1_b, W2_w, W2_b, V_w, V_b, U)

    in_maps = []
    for c in range(N_CORES):
        xs = x[c * BPC : (c + 1) * BPC]
        in_maps.append(
            {
                "xb": np.ascontiguousarray(xs).astype(ml_dtypes.bfloat16),
                "xT": np.ascontiguousarray(xs.transpose(0, 2, 1)),
                "hT": np.ascontiguousarray(hidden[c * BPC : (c + 1) * BPC].T).astype(ml_dtypes.bfloat16),
                "w1T": w1T,
                "w2T": w2T,
                "bsum": bsum,
                "vT": vT,
                "vb": vb,
            }
        )

    res = run_bass_kernel_spmd(nc, in_maps, list(range(N_CORES)), trace=TRACE)
    LAST["exec_time_ns"] = res.exec_time_ns
    LAST["results"] = res
    outs = res.results
    ctx = np.concatenate([r["ctx"] for r in outs], axis=0)[None, :, :]
    attw = np.concatenate([r["attw"] for r in outs], axis=0)[:, None, :]
    return ctx.astype(np.float32), attw.astype(np.float32)


# revision 43
# speedup vs baseline: 1.0269x; 1.0091x over previous
"""Bahdanau attention Trainium2 kernel, data-parallel over batch on 8 NeuronCores.

Computation (per batch b):
    att[t, v]  = tanh(sum_u x[t,u] W1[v,u] + hW2[b,v] + W1_b[v] + W2_b[v])
    score[t]   = sum_v V[v] att[t,v] + V_b
    e[t]       = exp(score[t])            (softmax without max-subtraction:
                                           |score| <= sum|V| + |V_b| ~ 33, safe in f32)
    w[t]       = e[t] / S,  S = sum_t e[t]
    ctx[u]     = sum_t e[t] x[t,u] / S

Device layout choices:
  - main matmul computes att TRANSPOSED: out1[v, t] (v on partitions) with
    lhsT = W1^T chunks [u,v] (float32r: full-rate PE at N>=256) and
    rhs = x^T chunks [u,t]; x^T is prepared on the host.
  - V-reduction: 4 col-tiled bf16 matmuls (tile_position) put partial scores
    at psum partitions 0/32/64/96; a ones-masked matmul folds them to one row.
  - e is reshaped [1,T] -> [128, T/128] chunkwise by SBUF->SBUF DMAs with
    t = p*NJ + j ordering, so e columns serve as matmul lhsT for the context
    accumulation against natural-layout x rows loaded j-strided (bf16).
  - context matmuls are col-tiled the same way; each batch's context phase is
    software-pipelined into the next batch's main phase.
"""

import os
from contextlib import ExitStack

import ml_dtypes
import numpy as np

import concourse.bass as bass
import concourse.tile as tile
from concourse import bacc, mybir
from concourse.bass_utils import run_bass_kernel_spmd

F32 = mybir.dt.float32
F32R = mybir.dt.float32r
BF16 = mybir.dt.bfloat16
FP16 = mybir.dt.float16
P = 128

B_FULL, T_FULL, U_FULL = 32, 2048, 1024
N_CORES = 8

# knobs for the local test harness (the grading path leaves these alone)
TRACE = False
LAST = {}


def build_nc(BPC, T, U):
    KC = U // P          # number of 128-wide u/v chunks
    NT = T // 512        # t-chunks for the main matmul
    NJ = T // P          # j-tiles for the context matmul (t = p*NJ + j)
    UCH = min(U, 512)    # u-chunk width for context output
    NU = U // UCH
    G = min(4, KC)       # col-tiles per V-MM round
    NR = KC // G         # V-MM rounds per t-chunk
    GJ = min(4, NJ)      # col-tiles per context round
    NRJ = NJ // GJ       # context rounds

    nc = bacc.Bacc(None, debug=False)

    xT_d = nc.dram_tensor("xT", [BPC, U, T], F32R, kind="ExternalInput")
    x_d = nc.dram_tensor("xb", [BPC, T, U], BF16, kind="ExternalInput")
    hT_d = nc.dram_tensor("hT", [U, BPC], BF16, kind="ExternalInput")
    w1T_d = nc.dram_tensor("w1T", [U, U], F32R, kind="ExternalInput")
    w2T_d = nc.dram_tensor("w2T", [U, U], BF16, kind="ExternalInput")
    bsum_d = nc.dram_tensor("bsum", [P, KC], F32, kind="ExternalInput")
    vT_d = nc.dram_tensor("vT", [P, KC], BF16, kind="ExternalInput")
    vb_d = nc.dram_tensor("vb", [1, 1], F32, kind="ExternalInput")
    ctx_d = nc.dram_tensor("ctx", [BPC, U], F32, kind="ExternalOutput")
    attw_d = nc.dram_tensor("attw", [BPC, T], F32, kind="ExternalOutput")

    Tanh = mybir.ActivationFunctionType.Tanh
    Exp = mybir.ActivationFunctionType.Exp

    with tile.TileContext(nc) as tc:
        with (
            tc.tile_pool(name="consts", bufs=1) as consts,
            tc.tile_pool(name="w1", bufs=1) as w1p,
            tc.tile_pool(name="xt", bufs=24) as xtp,
            tc.tile_pool(name="xn", bufs=NJ + 6) as xnp,
            tc.tile_pool(name="att", bufs=7) as attp,
            tc.tile_pool(name="e", bufs=2) as ep,
            tc.tile_pool(name="outs", bufs=1) as outp,
        ):
            vT_sb = consts.tile([P, KC], BF16)
            nc.sync.dma_start(out=vT_sb, in_=vT_d[:, :])
            bsum_sb = consts.tile([P, KC], F32)
            nc.sync.dma_start(out=bsum_sb, in_=bsum_d[:, :])
            vb_sb = consts.tile([1, 1], F32)
            nc.sync.dma_start(out=vb_sb, in_=vb_d[:, :])
            hT_sb = consts.tile([P, KC, BPC], BF16)
            nc.sync.dma_start(out=hT_sb, in_=hT_d.rearrange("(k p) b -> p k b", p=P))
            cT_sb = consts.tile([P, KC, BPC], F32)
            # ones at partitions 0,32,... reduce col-tiled psum partials with
            # one matmul instead of a single-lane DVE add chain
            ones_j = consts.tile([P, 1], BF16)
            nc.vector.memset(ones_j, 0.0)
            for i in range(GJ):
                nc.vector.memset(ones_j[32 * i : 32 * i + 1, :], 1.0)
            ones_v = consts.tile([P, 1], FP16)
            nc.vector.memset(ones_v, 0.0)
            for i in range(G):
                nc.vector.memset(ones_v[32 * i : 32 * i + 1, :], 1.0)

            # prologue: cT[p, c, b] = (hidden_b @ W2^T)[c*128+p] + W1_b + W2_b
            # W2^T streams in k-slabs; all KC psum accumulators live at once
            # (prologue-only, so the full 8 banks are free).
            with (
                tc.tile_pool(name="w2", bufs=2) as w2p,
                tc.tile_pool(name="psH", bufs=1, space=bass.MemorySpace.PSUM) as psh,
            ):
                phs = [
                    psh.tile([P, BPC], F32, name=f"ph{c}", tag=f"ph{c}")
                    for c in range(KC)
                ]
                for k in range(KC):
                    w2s = w2p.tile([P, U], BF16)
                    nc.sync.dma_start(
                        out=w2s, in_=w2T_d[k * P : (k + 1) * P, :]
                    )
                    for c in range(KC):
                        nc.tensor.matmul(
                            phs[c],
                            (w2s[:, c * P : (c + 1) * P]),
                            (hT_sb[:, k, :]),
                            start=(k == 0),
                            stop=(k == KC - 1),
                        )
                for c in range(KC):
                    nc.vector.tensor_scalar_add(
                        cT_sb[:, c, :], phs[c], bsum_sb[:, c : c + 1]
                    )

            # batch-0 t-chunk-0 x loads, then the W1 slabs: this DMA order
            # lets the first main matmuls start as soon as slab 0 lands
            # (the first t-chunk runs k-outer), instead of waiting for the
            # whole 4MB of W1 behind the prologue stream.
            xk00 = []
            for k in range(KC):
                xk = xtp.tile([P, 512], F32R, name="xk", tag="xk")
                nc.sync.dma_start(out=xk, in_=xT_d[0, k * P : (k + 1) * P, 0:512])
                xk00.append(xk)
            w1_sb = w1p.tile([P, KC, U], F32R)
            for k in range(KC):
                nc.sync.dma_start(
                    out=w1_sb[:, k, :], in_=w1T_d[k * P : (k + 1) * P, :]
                )

            with (
                tc.tile_pool(name="psA", bufs=4, space=bass.MemorySpace.PSUM) as ps_main,
                tc.tile_pool(name="psS", bufs=2, space=bass.MemorySpace.PSUM) as ps_sc,
                tc.tile_pool(name="psC", bufs=1, space=bass.MemorySpace.PSUM) as ps_ctx,
            ):
                pe_defer = []         # PE work emitted one v-chunk late so it
                                      # never stalls on a just-issued tanh

                def emit_main_tchunk(b, t, st, xks=None):
                    """xk loads + main MMs + col-tiled V-MMs + exp for one
                    t-chunk; xn prefetch for the context phase is interleaved
                    so the first t-chunk's xk loads are never queued behind
                    it."""
                    if xks is None:
                        xks = []
                        for k in range(KC):
                            xk = xtp.tile([P, 512], F32R)
                            nc.sync.dma_start(
                                out=xk,
                                in_=xT_d[b, k * P : (k + 1) * P,
                                         t * 512 : (t + 1) * 512],
                            )
                            xks.append(xk)
                    jlo = (t - 1 if b == 0 else t) * NJ // NT
                    jhi = (t if b == 0 else t + 1) * NJ // NT
                    for j in range(max(jlo, 0), jhi):
                        xn = xnp.tile([P, U], BF16)
                        nc.sync.dma_start(
                            out=xn, in_=x_d[b].rearrange("(p j) u -> j p u", j=NJ)[j]
                        )
                        st["xn"].append(xn)
                    # quad-partial scores: col-tile i writes partition 32*i
                    sc4 = ps_sc.tile([P, 512], F32)
                    # the very last chunk folds via full-tile copy + ones-MM
                    # (shorter latency); that path reads all partitions
                    last_chunk = (b == BPC - 1 and t == NT - 1)
                    if last_chunk:
                        nc.vector.memset(sc4, 0.0)
                    atts = []

                    def v_round(r):
                        for i in range(G):
                            nc.tensor.matmul(
                                sc4[32 * i : 32 * i + 1, :],
                                vT_sb[:, r * G + i : r * G + i + 1],
                                atts[r * G + i],
                                start=(r == 0),
                                stop=(r == NR - 1),
                                tile_position=(0, 32 * i),
                            )

                    def emit_v_chunk_tail(v):
                        if pe_defer:
                            pe_defer.pop(0)()
                        if (v + 1) % G == 0:
                            r = (v + 1) // G - 1
                            pe_defer.append(lambda r=r: v_round(r))

                    for v in range(KC):
                        o1 = ps_main.tile([P, 512], F32, name="o1", tag="o1")
                        for k in range(KC):
                            nc.tensor.matmul(
                                o1,
                                w1_sb[:, k, v * P : (v + 1) * P],
                                xks[k],
                                start=(k == 0),
                                stop=(k == KC - 1),
                            )
                        att = attp.tile([P, 512], BF16, name="att", tag="att")
                        nc.scalar.activation(
                            att, o1, Tanh, bias=cT_sb[:, v, b : b + 1]
                        )
                        atts.append(att)
                        emit_v_chunk_tail(v)

                    def reduce_and_exp():
                        if last_chunk:
                            # fast fold: this chain gates the final context
                            # matmuls, so spend one PE MM to shorten it
                            s_cp = attp.tile(
                                [P, 512], FP16, name="s_cp", tag="s_cp", bufs=1
                            )
                            nc.vector.tensor_copy(s_cp, sc4)
                            nc.tensor.matmul(
                                sc4[0:1, :], ones_v, s_cp, start=True, stop=True
                            )
                            s_red = sc4[0:1, :]
                        else:
                            # fold the G partial score rows (DVE; off the PE
                            # critical path), then exponentiate
                            s_sb = ep.tile([1, 512], F32, name="s_sb", tag="s_sb")
                            nc.vector.tensor_copy(s_sb, sc4[0:1, :])
                            for i in range(1, G):
                                nc.vector.tensor_add(
                                    s_sb, s_sb, sc4[32 * i : 32 * i + 1, :]
                                )
                            s_red = s_sb
                        nc.scalar.activation(
                            st["e_row"][:, t * 512 : (t + 1) * 512],
                            s_red,
                            Exp,
                            bias=vb_sb[0:1, :],
                            accum_out=st["s_part"][0:1, t : t + 1],
                        )
                        # incremental e reshape: this t-chunk covers rows
                        # 512//NJ * t .. of e_col (t' = p*NJ + j)
                        RPC = 512 // NJ
                        nc.sync.dma_start(
                            out=st["e_col"][RPC * t : RPC * (t + 1), :],
                            in_=st["e_row"][
                                :, t * 512 : (t + 1) * 512
                            ].rearrange("a (p j) -> a p j", j=NJ),
                        )

                    pe_defer.append(reduce_and_exp)

                def emit_tail(b, st):
                    """softmax: S, 1/S, normalized weights out, e reshaped."""
                    if b == 0:
                        for j in range((NT - 1) * NJ // NT, NJ):
                            xn = xnp.tile([P, U], BF16)
                            nc.sync.dma_start(
                                out=xn,
                                in_=x_d[b].rearrange("(p j) u -> j p u", j=NJ)[j],
                            )
                            st["xn"].append(xn)
                    for fn in pe_defer:
                        fn()
                    pe_defer.clear()
                    s_tot = ep.tile([1, 1], F32)
                    nc.vector.tensor_reduce(
                        s_tot, st["s_part"][0:1, :], axis=mybir.AxisListType.X,
                        op=mybir.AluOpType.add,
                    )
                    rec = ep.tile([1, 1], F32)
                    nc.vector.reciprocal(rec, s_tot)
                    st["rec"] = rec

                    w_sb = outp.tile([1, T], F32)
                    nc.vector.tensor_scalar_mul(
                        w_sb, st["e_row"][0:1, :], rec[0:1, 0:1]
                    )
                    nc.sync.dma_start(out=attw_d[b : b + 1, :], in_=w_sb)


                def emit_ctx(b, st):
                    """context: ctx[u] = (sum_t e[t] x[t,u]) / S, with 4
                    j-tiles col-tiled per round (partials at partition 32*i)"""
                    cps = [
                        ps_ctx.tile([P, UCH], F32, name=f"cps{uh}", tag=f"cps{uh}")
                        for uh in range(NU)
                    ]
                    for uh in range(NU):
                        nc.vector.memset(cps[uh], 0.0)
                    for r in range(NRJ):
                        for uh in range(NU):
                            for i in range(GJ):
                                j = r * GJ + i
                                nc.tensor.matmul(
                                    cps[uh][32 * i : 32 * i + 1, :],
                                    st["e_col"][:, j : j + 1],
                                    st["xn"][j][:, uh * UCH : (uh + 1) * UCH],
                                    start=(r == 0),
                                    stop=(r == NRJ - 1),
                                    tile_position=(0, 32 * i),
                                )
                    ctx_sb = outp.tile([1, U], F32)
                    for uh in range(NU):
                        cp_sb = attp.tile(
                            [P, UCH], BF16, name="cp_sb", tag="cp_sb", bufs=2
                        )
                        nc.vector.tensor_copy(cp_sb, cps[uh])
                        nc.tensor.matmul(
                            cps[uh][0:1, :], ones_j, cp_sb, start=True, stop=True
                        )
                        cs = ctx_sb[0:1, uh * UCH : (uh + 1) * UCH]
                        nc.vector.tensor_scalar_mul(
                            cs, cps[uh][0:1, :], st["rec"][0:1, 0:1]
                        )
                    nc.sync.dma_start(out=ctx_d[b : b + 1, :], in_=ctx_sb)

                prev = None
                for b in range(BPC):
                    st = {
                        "xn": [],
                        "e_row": ep.tile([1, T], BF16, name="e_row", tag="e_row"),
                        "e_col": ep.tile([P, NJ], BF16, name="e_col", tag="e_col"),
                        "s_part": ep.tile([1, NT], F32, name="s_part", tag="s_part"),
                    }
                    for t in range(NT):
                        emit_main_tchunk(
                            b, t, st,
                            xks=xk00 if (b == 0 and t == 0) else None,
                        )
                        if t == 0 and prev is not None:
                            # previous batch's context MMs slot in here, where
                            # their e_col/xn dependencies are long satisfied
                            emit_ctx(b - 1, prev)
                            prev = None
                    emit_tail(b, st)
                    prev = st
                for fn in pe_defer:
                    fn()
                pe_defer.clear()
                emit_ctx(BPC - 1, prev)

    nc.compile()
    return nc


_NC_CACHE = {}


def _get_nc(BPC, T, U):
    key = (BPC, T, U)
    if key not in _NC_CACHE:
        _NC_CACHE[key] = build_nc(BPC, T, U)
    return _NC_CACHE[key]


def _prep_shared(W1_w, W1_b, W2_w, W2_b, V_w, V_b, U):
    KC = U // P
    w1T = np.ascontiguousarray(W1_w.T)
    w2T = np.ascontiguousarray(W2_w.T).astype(ml_dtypes.bfloat16)
    bsum = np.ascontiguousarray((W1_b + W2_b).reshape(KC, P).T)
    vT = np.ascontiguousarray(V_w.reshape(KC, P).T).astype(ml_dtypes.bfloat16)
    vb = np.asarray(V_b, np.float32).reshape(1, 1)
    return w1T, w2T, bsum, vT, vb


def kernel(x, hidden, W1_w, W1_b, W2_w, W2_b, V_w, V_b):
    x = np.asarray(x, np.float32)
    hidden = np.asarray(hidden, np.float32)
    W1_w = np.asarray(W1_w, np.float32)
    W1_b = np.asarray(W1_b, np.float32)
    W2_w = np.asarray(W2_w, np.float32)
    W2_b = np.asarray(W2_b, np.float32)
    V_w = np.asarray(V_w, np.float32)
    V_b = np.asarray(V_b, np.float32)

    if TRACE:
        os.environ.pop("BASS_NEVER_TRACE", None)
    else:
        # the axon trace path needs a profiling hook this image may lack;
        # make sure an ambient BASS_TRACE can't route us into it
        os.environ["BASS_NEVER_TRACE"] = "1"

    B, T, U = x.shape
    BPC = B // N_CORES
    nc = _get_nc(BPC, T, U)
    w1T, w2T, bsum, vT, vb = _prep_shared(W1_w, W1_b, W2_w, W2_b, V_w, V_b, U)

    in_maps = []
    for c in range(N_CORES):
        xs = x[c * BPC : (c + 1) * BPC]
        in_maps.append(
            {
                "xb": np.ascontiguousarray(xs).astype(ml_dtypes.bfloat16),
                "xT": np.ascontiguousarray(xs.transpose(0, 2, 1)),
                "hT": np.ascontiguousarray(hidden[c * BPC : (c + 1) * BPC].T).astype(ml_dtypes.bfloat16),
                "w1T": w1T,
                "w2T": w2T,
                "bsum": bsum,
                "vT": vT,
                "vb": vb,
            }
        )

    res = run_bass_kernel_spmd(nc, in_maps, list(range(N_CORES)), trace=TRACE)
    LAST["exec_time_ns"] = res.exec_time_ns
    LAST["results"] = res
    outs = res.results
    ctx = np.concatenate([r["ctx"] for r in outs], axis=0)[None, :, :]
    attw = np.concatenate([r["attw"] for r in outs], axis=0)[:, None, :]
    return ctx.astype(np.float32), attw.astype(np.float32)
